# revision 4
# baseline (speedup 1.0000x reference)
"""NegLogLikelihood (masked BCE log-sum) on 8 Trainium2 NeuronCores.

Math: p = pred_hz[:, :, 0]; ll = sum(where(m, log(p), log1p(-p)));
out = -ll / BATCH.

Identity used on device: q = m ? p : (1-p) = |(p - 1) + m| for m in {0,1},
so each element costs one fused DVE op (p-1)+m, one ACT Abs, one ACT Ln
whose free accum_out gives the per-partition sum.

Sharding: data-parallel over batch. Core i gets rows [32i, 32(i+1)) of
channel 0 only (the other 7 channels are dead weight; host slicing avoids
an 8x-inefficient strided DMA). Host does the final tiny f64 reduction.
"""

import numpy as np

B, G, T = 256, 16384, 8
NCORES = 8
ROWS = B // NCORES          # 32 batch rows per core
P = 128                     # SBUF partitions
F = ROWS * G // P           # 4096 free elements per partition per core
CHUNK = 2048
NT = F // CHUNK

_cache = {}


def _build(repeat=1, trip=None):
    from concourse import bacc, mybir, tile
    from contextlib import nullcontext

    nc = bacc.Bacc(
        "TRN2",
        target_bir_lowering=False,
        debug=False,
        enable_asserts=False,
        num_devices=NCORES,
        enable_partition_id=False,
    )
    p_d = nc.dram_tensor("p", [P, F], mybir.dt.float32, kind="ExternalInput")
    m_d = nc.dram_tensor("m", [P, F], mybir.dt.uint8, kind="ExternalInput")
    out_d = nc.dram_tensor("partials", [P, NT], mybir.dt.float32,
                           kind="ExternalOutput")

    with tile.TileContext(nc) as tc:
        with tc.tile_pool(name="io", bufs=2) as pool, \
             tc.tile_pool(name="acc", bufs=1) as accpool:
            out_sb = accpool.tile([P, NT], mybir.dt.float32)
            loop_cm = tc.For_i(0, trip) if trip else nullcontext()
            with loop_cm:
                for _ in range(repeat):
                    for j in range(NT):
                        sl = slice(j * CHUNK, (j + 1) * CHUNK)
                        p_t = pool.tile([P, CHUNK], mybir.dt.float32, tag="p")
                        m_t = pool.tile([P, CHUNK], mybir.dt.uint8, tag="m")
                        nc.sync.dma_start(out=p_t, in_=p_d.ap()[:, sl])
                        nc.sync.dma_start(out=m_t, in_=m_d.ap()[:, sl])
                        e_t = pool.tile([P, CHUNK], mybir.dt.float32, tag="e")
                        nc.vector.scalar_tensor_tensor(
                            out=e_t, in0=p_t, scalar=-1.0, in1=m_t,
                            op0=mybir.AluOpType.add, op1=mybir.AluOpType.add,
                        )
                        a_t = pool.tile([P, CHUNK], mybir.dt.float32, tag="a")
                        nc.scalar.activation(
                            out=a_t, in_=e_t,
                            func=mybir.ActivationFunctionType.Abs,
                        )
                        l_t = pool.tile([P, CHUNK], mybir.dt.float32, tag="l")
                        nc.scalar.activation(
                            out=l_t, in_=a_t,
                            func=mybir.ActivationFunctionType.Ln,
                            accum_out=out_sb[:, j:j + 1],
                        )
            nc.sync.dma_start(out=out_d.ap(), in_=out_sb)
    nc.compile()
    return nc


def _in_maps(pred_hz, target_m):
    pred_hz = np.asarray(pred_hz)
    target_m = np.asarray(target_m)
    maps = []
    for i in range(NCORES):
        rows = slice(i * ROWS, (i + 1) * ROWS)
        p_i = np.ascontiguousarray(pred_hz[rows, :, 0]).reshape(P, F)
        m_i = (np.ascontiguousarray(target_m[rows])
               .view(np.uint8).reshape(P, F))
        maps.append({"p": p_i, "m": m_i})
    return maps


def _run(pred_hz, target_m, trace=False, **kw):
    from concourse import bass_utils

    if "nc" not in _cache:
        _cache["nc"] = _build()
    return bass_utils.run_bass_kernel_spmd(
        _cache["nc"], _in_maps(pred_hz, target_m),
        core_ids=list(range(NCORES)), trace=trace, **kw,
    )


def kernel(pred_hz: np.ndarray, target_m: np.ndarray) -> np.ndarray:
    res = _run(pred_hz, target_m)
    total = 0.0
    for r in res.results:
        total += float(np.sum(np.asarray(r["partials"], dtype=np.float64)))
    return np.array(-total / B, dtype=np.float32)
